# revision 1
# baseline (speedup 1.0000x reference)
"""MoE MLP (top-2 routing, 8 experts) on 8 Trainium2 NeuronCores.

Strategy (expert-parallel, per the sharding hint): each core owns one
expert's weights. The router (a [8,1024] matmul + softmax + top-2 —
0.05% of total FLOPs) runs on the host, which doubles as the dispatch
step: tokens are gathered per selected expert and shipped to that
expert's core, replacing the all-to-all. Each core runs a fused
gelu-MLP Bass kernel over its routed tokens:

    yT = w ⊙ (W_out^T @ gelu(W_in^T @ xT + b_in) + b_out)

in a transposed layout (tokens along the free axis) so both matmuls
keep the *weights* stationary on the PE array and no on-chip
transposes are needed anywhere. W_out stays resident in SBUF; W_in
streams once per token chunk. The host scatter-adds the per-expert
results back into the full [B,S,D] output.

Matmuls run in fp16 (same PE throughput as bf16 — 4x fp32 — but 8x
finer mantissa; measured end-to-end error vs the fp32 reference is
~4e-4 scale-relative). Set MOE_PREC=fp32 to force full fp32 matmuls.
"""

import contextlib
import ctypes
import os
import sys
import types
from contextlib import ExitStack

import numpy as np

import concourse.bass as bass
import concourse.mybir as mybir
import concourse.tile as tile
from concourse import bacc
from concourse.bass_utils import run_bass_kernel_spmd


def _install_ntff_hook():
    """Provide antenv.axon_hooks (absent in this image) so BASS_TRACE=1
    can capture NTFF profiles through the axon PJRT .so. No-op if the
    module already exists or the .so/symbols are unavailable."""
    try:
        from antenv.axon_hooks import get_axon_ntff_profile_hook  # noqa: F401
        return
    except ImportError:
        pass
    so_path = "/opt/axon/libaxon_pjrt.so"
    if not os.path.exists(so_path):
        return
    try:
        lib = ctypes.CDLL(so_path)
    except OSError:
        return
    if not hasattr(lib, "axon_start_nrt_profile"):
        return
    lib.axon_start_nrt_profile.argtypes = [
        ctypes.POINTER(ctypes.c_int64), ctypes.c_size_t]
    lib.axon_start_nrt_profile.restype = ctypes.c_int64
    lib.axon_stop_nrt_profile.argtypes = [ctypes.c_char_p]
    lib.axon_stop_nrt_profile.restype = ctypes.c_int64

    @contextlib.contextmanager
    def _hook(output_dir, device_ids):
        import jax
        jax.devices()  # force PJRT init so the .so's client exists
        if device_ids:
            ids = (ctypes.c_int64 * len(device_ids))(*device_ids)
            rc = lib.axon_start_nrt_profile(ids, len(device_ids))
        else:
            rc = lib.axon_start_nrt_profile(None, 0)
        if rc != 0:
            raise RuntimeError(f"axon_start_nrt_profile rc={rc}")
        try:
            yield
        finally:
            n = lib.axon_stop_nrt_profile(str(output_dir).encode())
            print(f"ntff profile: {n} file(s) -> {output_dir}", file=sys.stderr)

    import antenv
    mod = types.ModuleType("antenv.axon_hooks")
    mod.get_axon_ntff_profile_hook = lambda: _hook
    mod.set_axon_ntff_profile_hook = lambda h: None
    sys.modules["antenv.axon_hooks"] = mod
    antenv.axon_hooks = mod

B, S, D, F, E = 4, 2048, 1024, 4096, 8
T = B * S
TOP_K = 2
NCORES = 8
P = 128
ND, NF = D // P, F // P  # 8, 32

# test.py pokes these for profiling info
LAST_RESULT = None

_cache = {}


def _chunk_list(C):
    """Token chunks (PSUM free-dim <= 512, multiples of 128).

    Chunks below 256 run LDWEIGHTS-bound on the PE (weight load ~60ns
    vs a 53ns N=128 matmul), so a short tail is split off the previous
    512 chunk into two >=256 pieces instead.
    """
    chunks = [512] * (C // 512)
    rem = C % 512
    if rem:
        if rem < 256 and chunks:
            total = 512 + rem
            a = ((total // 2 + 127) // 128) * 128
            chunks[-1] = a
            chunks.append(total - a)
        else:
            chunks.append(rem)
    return chunks


def _build_bass(C, prec):
    dt = mybir.dt
    fp16_path = prec != "fp32"
    io_dt = dt.float16 if fp16_path else dt.float32
    nc = bacc.Bacc("TRN2", target_bir_lowering=False, debug=False)

    xT = nc.dram_tensor("xT", [D, C], io_dt, kind="ExternalInput")
    win = nc.dram_tensor("win", [D, F], io_dt, kind="ExternalInput")
    wout = nc.dram_tensor("wout", [F, D], io_dt, kind="ExternalInput")
    bin_ = nc.dram_tensor("bin", [F], dt.float32, kind="ExternalInput")
    bout = nc.dram_tensor("bout", [D], dt.float32, kind="ExternalInput")
    wcomb = nc.dram_tensor("wcomb", [P, C], dt.float32, kind="ExternalInput")
    yT = nc.dram_tensor("yT", [D, C], dt.float32, kind="ExternalOutput")

    xT_r = xT.ap().rearrange("(dn p) c -> p dn c", p=P)
    win_r = win.ap().rearrange("(dn p) f -> p dn f", p=P)
    wout_r = wout.ap().rearrange("(fn p) d -> p fn d", p=P)
    yT_r = yT.ap().rearrange("(dn p) c -> p dn c", p=P)

    chunks = _chunk_list(C)

    with tile.TileContext(nc) as tc, ExitStack() as ctx:
        consts = ctx.enter_context(tc.tile_pool(name="consts", bufs=1))
        xpool = ctx.enter_context(tc.tile_pool(name="x", bufs=2))
        winpool = ctx.enter_context(tc.tile_pool(name="win", bufs=3))
        woutpool = ctx.enter_context(tc.tile_pool(name="wout", bufs=1))
        hpool = ctx.enter_context(tc.tile_pool(name="h", bufs=1))
        ypool = ctx.enter_context(tc.tile_pool(name="y", bufs=4))
        psum_h = ctx.enter_context(tc.tile_pool(name="ph", bufs=4, space="PSUM"))
        psum_y = ctx.enter_context(tc.tile_pool(name="py", bufs=2, space="PSUM"))

        def x_dma(ck, csl):
            x_t = xpool.tile([P, ND, ck], io_dt, tag="x")
            nc.sync.dma_start(x_t[:], xT_r[:, :, csl])
            return x_t

        def win_dma(fo):
            win_t = winpool.tile([P, ND, 512], io_dt, tag="win")
            nc.sync.dma_start(win_t[:], win_r[:, :, fo * 512:(fo + 1) * 512])
            return win_t

        # critical path for the very first matmul: x chunk 0 + W_in
        # stripe 0 go FIRST, each split across BOTH HWDGE queues (Sync +
        # Act) — a single dma_start runs ~150 GB/s, so two in parallel
        # roughly halve the time to first matmul.
        ck0 = chunks[0]
        x0_t = xpool.tile([P, ND, ck0], io_dt, tag="x")
        nc.sync.dma_start(x0_t[:, :4, :], xT_r[:, :4, slice(0, ck0)])
        nc.scalar.dma_start(x0_t[:, 4:, :], xT_r[:, 4:, slice(0, ck0)])
        win0_t = winpool.tile([P, ND, 512], io_dt, tag="win")
        nc.sync.dma_start(win0_t[:, :4, :], win_r[:, :4, 0:512])
        nc.scalar.dma_start(win0_t[:, 4:, :], win_r[:, 4:, 0:512])

        # b_in is needed by the first gelu; it's tiny — SWDGE queue.
        bin_t = consts.tile([P, NF], dt.float32)
        nc.gpsimd.dma_start(bin_t[:], bin_.ap().rearrange("(fo fi) -> fi fo", fi=P))

        # PE HAM warm-up: ~3us of junk matmuls on a scratch tile while the
        # x0/win0 DMAs are in flight, so real matmuls start at 2.4 GHz
        # instead of spending the first activity window at 1.2 GHz.
        wu_t = consts.tile([P, P], io_dt)
        nc.gpsimd.memset(wu_t[:], 0.0)
        wu_ps = ctx.enter_context(tc.tile_pool(name="wups", bufs=1, space="PSUM"))
        wu_p = wu_ps.tile([P, 64], dt.float32)
        for _ in range(60):
            nc.tensor.matmul(wu_p[:], wu_t[:], wu_t[:, :64], start=True, stop=True)

        # Remaining bulk loads share the Sync HWDGE queue with the W_in
        # stripes, hand-interleaved below so each arrives just in time:
        # the queue drains in emission order, so wout stripe k loads
        # during phase-A stripe k's ~7us of matmuls and the whole of
        # W_out is resident right when phase B first needs it. (Putting
        # them on another queue doesn't work: the scheduler hoists
        # ready DMA triggers, and they'd steal HBM bandwidth from the
        # critical x0/win0 loads.)
        bout_t = consts.tile([P, ND], dt.float32)
        w_t = consts.tile([P, C], dt.float32)
        wout_tiles = []
        if fp16_path:
            for fo in range(8):
                wout_tiles.append(
                    woutpool.tile([P, 4, D], io_dt,
                                  tag=f"wout{fo}", name=f"wout{fo}"))

        off = 0
        for ci, ck in enumerate(chunks):
            csl = slice(off, off + ck)
            x_t = x0_t if ci == 0 else x_dma(ck, csl)

            # ---- phase A: h = gelu(W_in^T @ x + b_in), laid out [f, tok]
            h_t = hpool.tile([P, NF, ck], io_dt, tag="h")
            for fo in range(8):  # 512-wide stripes of F
                win_t = win0_t if (ci == 0 and fo == 0) else win_dma(fo)
                for j in range(4):
                    fc = fo * 4 + j
                    ph = psum_h.tile([P, ck], dt.float32, tag="ph")
                    for dn in range(ND):
                        nc.tensor.matmul(
                            ph[:],
                            win_t[:, dn, j * P:(j + 1) * P],
                            x_t[:, dn, :],
                            start=(dn == 0),
                            stop=(dn == ND - 1),
                        )
                    nc.scalar.activation(
                        h_t[:, fc, :], ph[:],
                        mybir.ActivationFunctionType.Gelu,
                        bias=bin_t[:, fc:fc + 1],
                    )
                if ci == 0:
                    if fp16_path:
                        # interleave the resident W_out load with the
                        # W_in stream: stripe fo rides the queue behind
                        # win stripe fo, loading during its ~7us of
                        # matmuls, so W_out has landed by phase B.
                        nc.sync.dma_start(
                            wout_tiles[fo][:],
                            wout_r[:, fo * 4:(fo + 1) * 4, :])
                    if fo == 3:
                        nc.sync.dma_start(
                            bout_t[:],
                            bout.ap().rearrange("(do di) -> di do", di=P))
                    elif fo == 5:
                        nc.sync.dma_start(w_t[:], wcomb.ap())

            # ---- phase B: y = w * (W_out^T @ h + b_out), laid out [d, tok]
            if fp16_path:
                for dn in range(ND):
                    py = psum_y.tile([P, ck], dt.float32, tag="py")
                    for fc in range(NF):
                        nc.tensor.matmul(
                            py[:],
                            wout_tiles[fc // 4][:, fc % 4, dn * P:(dn + 1) * P],
                            h_t[:, fc, :],
                            start=(fc == 0),
                            stop=(fc == NF - 1),
                        )
                    y_t = ypool.tile([P, ck], dt.float32, tag="y")
                    # one DVE op: (psum + b_out) * w — keeps ScalarE on
                    # gelu only (no ACT table switching per chunk)
                    nc.vector.scalar_tensor_tensor(
                        y_t[:], py[:], bout_t[:, dn:dn + 1], w_t[:, csl],
                        op0=mybir.AluOpType.add, op1=mybir.AluOpType.mult,
                    )
                    nc.scalar.dma_start(yT_r[:, dn, csl], y_t[:])
            else:
                # fp32: W_out too big to keep resident; stream it per chunk
                # in two d-halves (4 PSUM banks live per half).
                for dh in range(2):
                    pys = []
                    for i in range(4):
                        py = psum_y.tile([P, ck], dt.float32, tag=f"py{i}")
                        pys.append(py)
                    for fc in range(NF):
                        wt = woutpool.tile([P, 512], io_dt, tag="wouts")
                        nc.sync.dma_start(
                            wt[:], wout_r[:, fc, dh * 512:(dh + 1) * 512])
                        for i in range(4):
                            nc.tensor.matmul(
                                py := pys[i],
                                wt[:, i * P:(i + 1) * P],
                                h_t[:, fc, :],
                                start=(fc == 0),
                                stop=(fc == NF - 1),
                            )
                    for i in range(4):
                        dn = dh * 4 + i
                        y_t = ypool.tile([P, ck], dt.float32, tag="y")
                        nc.scalar.activation(
                            y_t[:], pys[i][:],
                            mybir.ActivationFunctionType.Identity,
                            bias=bout_t[:, dn:dn + 1],
                        )
                        nc.vector.tensor_mul(y_t[:], y_t[:], w_t[:, csl])
                        nc.sync.dma_start(yT_r[:, dn, csl], y_t[:])
            off += ck

    nc.compile()
    return nc


def _get_nc(C, prec):
    key = (C, prec)
    if key not in _cache:
        _cache[key] = _build_bass(C, prec)
    return _cache[key]


def _route(x, W_router):
    """Host-side router: top-2 selection + renormalized weights (fp64).

    Matches jax.lax.top_k on softmax(logits): softmax is monotone so
    top-2 of logits is identical, with ties broken toward lower index
    (argsort stable on -logits).
    """
    lg = x.astype(np.float64) @ W_router.T.astype(np.float64)
    top2 = np.argsort(-lg, axis=1, kind="stable")[:, :TOP_K]
    l1 = np.take_along_axis(lg, top2[:, 0:1], 1)
    l2 = np.take_along_axis(lg, top2[:, 1:2], 1)
    e2 = np.exp(l2 - l1)
    w1 = (1.0 / (1.0 + e2)).astype(np.float32)
    w2 = (e2 / (1.0 + e2)).astype(np.float32)
    return top2, np.concatenate([w1, w2], axis=1)


def kernel(residual, W_router, W_in, b_in, W_out, b_out):
    global LAST_RESULT
    prec = os.environ.get("MOE_PREC", "fp16")
    np_io = np.float16 if prec != "fp32" else np.float32

    x = np.ascontiguousarray(np.asarray(residual, dtype=np.float32).reshape(T, D))
    W_in = np.asarray(W_in, dtype=np.float32)
    W_out = np.asarray(W_out, dtype=np.float32)
    b_in = np.asarray(b_in, dtype=np.float32)
    b_out = np.asarray(b_out, dtype=np.float32)

    top2, wts = _route(x, np.asarray(W_router, dtype=np.float32))

    idxs, ws = [], []
    for e in range(E):
        sel0 = top2[:, 0] == e
        sel1 = top2[:, 1] == e
        idx = np.concatenate([np.where(sel0)[0], np.where(sel1)[0]])
        w = np.concatenate([wts[sel0, 0], wts[sel1, 1]])
        idxs.append(idx)
        ws.append(w)

    C = max(len(i) for i in idxs)
    C = ((C + P - 1) // P) * P
    nc = _get_nc(C, prec)

    xt = np.ascontiguousarray(x.T)  # [D, T]
    in_maps = []
    for e in range(E):
        cnt = len(idxs[e])
        xT_e = np.zeros((D, C), dtype=np_io)
        xT_e[:, :cnt] = xt[:, idxs[e]]
        wc_e = np.zeros((P, C), dtype=np.float32)
        wc_e[:, :cnt] = ws[e][None, :]
        in_maps.append({
            "xT": xT_e,
            "win": np.ascontiguousarray(W_in[e], dtype=np_io),
            "wout": np.ascontiguousarray(W_out[e], dtype=np_io),
            "bin": b_in[e],
            "bout": b_out[e],
            "wcomb": wc_e,
        })

    if os.environ.get("BASS_TRACE"):
        _install_ntff_hook()
    LAST_RESULT = run_bass_kernel_spmd(nc, in_maps, list(range(NCORES)))

    y = np.zeros((T, D), dtype=np.float32)
    for e in range(E):
        cnt = len(idxs[e])
        y[idxs[e]] += LAST_RESULT.results[e]["yT"][:, :cnt].T
    return y.reshape(B, S, D)



# revision 4
# speedup vs baseline: 1.0479x; 1.0479x over previous
"""MoE MLP (top-2 routing, 8 experts) on 8 Trainium2 NeuronCores.

Strategy: expert-parallel along the *hidden* (F) axis instead of the
expert axis. The old one-expert-per-core split is load-imbalanced: the
busiest expert gets 2175 of the 16384 token-expert pairs while the
average is 2048, and exec time is the max over cores, so every core
pays the straggler's 6%. Here every core owns a 512-wide F-slice of
ALL 8 experts (W_in[:, :, sl], W_out[:, sl, :]) and processes ALL
routed tokens, producing a partial y that the host sums across cores.
Per-core work is identical by construction: 16384 tokens x (1024x512)
x 2 matmuls = 437us of fp16 PE roofline (vs 465us before).

The router (0.05% of FLOPs) runs on the host, which doubles as the
dispatch: tokens are gathered per selected expert into one [D, 16384]
fp16 stream shared by all cores. Each core runs the fused MLP

    y_partial = W_out[sl,:]^T @ gelu(W_in[:,sl]^T @ x + b_in[sl])

in a transposed layout (tokens on the free axis) so weights stay
stationary on the PE array. Combine weights and b_out are applied on
the host during the partial-sum reduction (free), so the device does
no per-token scaling at all. Weight slices stream per-expert (2MB
each) on the sync DMA ring two experts ahead; x chunks ride the
scalar ring; y partials (fp16) interleave on the sync ring.

fp8 was evaluated and rejected: e4m3 quantization measures 4.6e-2
rel-max error on this problem (gate is 2e-2); every partial-fp8
scheme also fails. fp16 measures ~4e-4.
"""

import contextlib
import ctypes
import os
import sys
import types
from contextlib import ExitStack

import numpy as np

import concourse.bass as bass
import concourse.mybir as mybir
import concourse.tile as tile
from concourse import bacc
from concourse.bass_utils import run_bass_kernel_spmd


def _install_ntff_hook():
    """Provide antenv.axon_hooks (absent in this image) so BASS_TRACE=1
    can capture NTFF profiles through the axon PJRT .so. No-op if the
    module already exists or the .so/symbols are unavailable."""
    try:
        from antenv.axon_hooks import get_axon_ntff_profile_hook  # noqa: F401
        return
    except ImportError:
        pass
    so_path = "/opt/axon/libaxon_pjrt.so"
    if not os.path.exists(so_path):
        return
    try:
        lib = ctypes.CDLL(so_path)
    except OSError:
        return
    if not hasattr(lib, "axon_start_nrt_profile"):
        return
    lib.axon_start_nrt_profile.argtypes = [
        ctypes.POINTER(ctypes.c_int64), ctypes.c_size_t]
    lib.axon_start_nrt_profile.restype = ctypes.c_int64
    lib.axon_stop_nrt_profile.argtypes = [ctypes.c_char_p]
    lib.axon_stop_nrt_profile.restype = ctypes.c_int64

    @contextlib.contextmanager
    def _hook(output_dir, device_ids):
        import jax
        jax.devices()  # force PJRT init so the .so's client exists
        if device_ids:
            ids = (ctypes.c_int64 * len(device_ids))(*device_ids)
            rc = lib.axon_start_nrt_profile(ids, len(device_ids))
        else:
            rc = lib.axon_start_nrt_profile(None, 0)
        if rc != 0:
            raise RuntimeError(f"axon_start_nrt_profile rc={rc}")
        try:
            yield
        finally:
            n = lib.axon_stop_nrt_profile(str(output_dir).encode())
            print(f"ntff profile: {n} file(s) -> {output_dir}", file=sys.stderr)

    import antenv
    mod = types.ModuleType("antenv.axon_hooks")
    mod.get_axon_ntff_profile_hook = lambda: _hook
    mod.set_axon_ntff_profile_hook = lambda h: None
    sys.modules["antenv.axon_hooks"] = mod
    antenv.axon_hooks = mod

B, S, D, F, E = 4, 2048, 1024, 4096, 8
T = B * S
TOP_K = 2
NCORES = 8
P = 128
FS = F // NCORES          # 512-wide F-slice per core
ND, NB = D // P, FS // P  # 8 d-tiles, 4 f-tiles per slice

# test.py pokes these for profiling info
LAST_RESULT = None

_cache = {}


def _chunk_list(C):
    """Token chunks (PSUM free-dim <= 512). Chunks below 256 run
    LDWEIGHTS-bound on the PE, so a short tail is split off the
    previous 512 chunk into two >=256 pieces instead."""
    chunks = [512] * (C // 512)
    rem = C % 512
    if rem:
        if rem < 256 and chunks:
            tot = 512 + rem
            a = tot // 2
            chunks[-1] = a
            chunks.append(tot - a)
        else:
            chunks.append(rem)
    return chunks


def _chunk_plan(counts):
    """[(expert, global_off, size, first_of_expert)], with a small ramp
    chunk up front (fast time-to-first-matmul) and a small tail chunk
    (short drain after the last matmul)."""
    lists = [_chunk_list(c) for c in counts]
    for lst in lists:
        if lst:
            if lst[0] >= 512:  # startup ramp: 256+256 instead of 512
                lst[0] = 256
                lst.insert(1, 256)
            break
    for lst in reversed(lists):
        if lst:
            if lst[-1] > 192:  # tail: split off a final 128
                lst[-1] -= 128
                lst.append(128)
            break
    plan = []
    off = 0
    for e, lst in enumerate(lists):
        for i, ck in enumerate(lst):
            plan.append((e, off, ck, i == 0))
            off += ck
    return plan, off


def _build_bass(counts):
    dt = mybir.dt
    io_dt = dt.float16
    plan, CT = _chunk_plan(counts)
    nc = bacc.Bacc("TRN2", target_bir_lowering=False, debug=False)

    x8 = nc.dram_tensor("x8", [D, CT], io_dt, kind="ExternalInput")
    win8 = nc.dram_tensor("win8", [D, E, FS], io_dt, kind="ExternalInput")
    wout8 = nc.dram_tensor("wout8", [FS, E, D], io_dt, kind="ExternalInput")
    bin8 = nc.dram_tensor("bin8", [E, FS], dt.float32, kind="ExternalInput")
    y8 = nc.dram_tensor("y8", [D, CT], io_dt, kind="ExternalOutput")

    x_r = x8.ap().rearrange("(dn p) c -> p dn c", p=P)
    win_r = win8.ap().rearrange("(dn p) e f -> p dn e f", p=P)
    wout_r = wout8.ap().rearrange("(fb p) e d -> p fb e d", p=P)
    bin_r = bin8.ap().rearrange("e (fc p) -> p e fc", p=P)
    y_r = y8.ap().rearrange("(dn p) c -> p dn c", p=P)

    with tile.TileContext(nc) as tc, ExitStack() as ctx:
        consts = ctx.enter_context(tc.tile_pool(name="consts", bufs=1))
        xpool = ctx.enter_context(tc.tile_pool(name="x", bufs=3))
        winpool = ctx.enter_context(tc.tile_pool(name="win", bufs=3))
        woutpool = ctx.enter_context(tc.tile_pool(name="wout", bufs=3))
        hpool = ctx.enter_context(tc.tile_pool(name="h", bufs=2))
        ypool = ctx.enter_context(tc.tile_pool(name="y", bufs=16))
        psum_h = ctx.enter_context(tc.tile_pool(name="ph", bufs=3, space="PSUM"))
        psum_y = ctx.enter_context(tc.tile_pool(name="py", bufs=3, space="PSUM"))

        def x_dma(e_off_ck):
            _, off, ck, _ = e_off_ck
            x_t = xpool.tile([P, ND, ck], io_dt, tag="x")
            nc.scalar.dma_start(x_t[:], x_r[:, :, off:off + ck])
            return x_t

        def w_dma(e):
            win_t = winpool.tile([P, ND, FS], io_dt, tag="win")
            nc.sync.dma_start(win_t[:], win_r[:, :, e, :])
            wout_t = woutpool.tile([P, NB, D], io_dt, tag="wout")
            nc.sync.dma_start(wout_t[:], wout_r[:, :, e, :])
            return win_t, wout_t

        # Critical path: x chunk 0 (scalar ring) and expert 0's W_in
        # (sync ring, dn-halves so the first matmul can start after
        # half the load) go first on their rings.
        x0_t = xpool.tile([P, ND, plan[0][2]], io_dt, tag="x", name="x0")
        nc.scalar.dma_start(x0_t[:], x_r[:, :, 0:plan[0][2]])
        x_pref = [x0_t]
        win0_t = winpool.tile([P, ND, FS], io_dt, tag="win")
        nc.sync.dma_start(win0_t[:, :4, :], win_r[:, :4, 0, :])
        nc.sync.dma_start(win0_t[:, 4:, :], win_r[:, 4:, 0, :])
        wout0_t = woutpool.tile([P, NB, D], io_dt, tag="wout")
        nc.sync.dma_start(wout0_t[:], wout_r[:, :, 0, :])
        w_tiles = {0: (win0_t, wout0_t)}

        # b_in is tiny and needed by the first gelu — SWDGE queue.
        bin_t = consts.tile([P, E, NB], dt.float32)
        nc.gpsimd.dma_start(bin_t[:], bin_r)

        # PE HAM warm-up: junk matmuls while the x0/win0 DMAs are in
        # flight, so real matmuls start at 2.4 GHz instead of 1.2.
        wu_t = consts.tile([P, P], io_dt)
        nc.vector.memset(wu_t[:], 0.0)
        wu_ps = ctx.enter_context(tc.tile_pool(name="wups", bufs=1, space="PSUM"))
        wu_p = wu_ps.tile([P, 64], dt.float32)
        for _ in range(56):
            nc.tensor.matmul(wu_p[:], wu_t[:], wu_t[:, :64], start=True, stop=True)

        # Prefetch: x one chunk ahead, weights one expert ahead.
        if len(plan) > 1:
            x_pref.append(x_dma(plan[1]))
        if E > 1 and counts[1]:
            w_tiles[1] = w_dma(1)

        for ci, (e, off, ck, first) in enumerate(plan):
            x_t = x_pref.pop(0)
            if ci + 2 < len(plan):
                x_pref.append(x_dma(plan[ci + 2]))
            if not first and e + 2 < E and counts[e + 2] and (e + 2) not in w_tiles \
                    and plan[ci - 1][3]:
                # second chunk of expert e: prefetch expert e+2's weights
                # (queued behind the first chunk's y DMAs on the sync ring)
                w_tiles[e + 2] = w_dma(e + 2)
            if e not in w_tiles:  # fallback for degenerate chunk plans
                w_tiles[e] = w_dma(e)
            win_t, wout_t = w_tiles[e]

            # ---- phase A: h = gelu(W_in^T @ x + b_in), laid out [f, tok]
            h_t = hpool.tile([P, NB, ck], io_dt, tag="h")
            for fc in range(NB):
                ph = psum_h.tile([P, ck], dt.float32, tag="ph")
                for dn in range(ND):
                    nc.tensor.matmul(
                        ph[:],
                        win_t[:, dn, fc * P:(fc + 1) * P],
                        x_t[:, dn, :],
                        start=(dn == 0),
                        stop=(dn == ND - 1),
                    )
                nc.scalar.activation(
                    h_t[:, fc, :], ph[:],
                    mybir.ActivationFunctionType.Gelu,
                    bias=bin_t[:, e, fc:fc + 1],
                )

            # ---- phase B: y_partial = W_out^T @ h, laid out [d, tok]
            for dn in range(ND):
                py = psum_y.tile([P, ck], dt.float32, tag="py")
                for fb in range(NB):
                    nc.tensor.matmul(
                        py[:],
                        wout_t[:, fb, dn * P:(dn + 1) * P],
                        h_t[:, fb, :],
                        start=(fb == 0),
                        stop=(fb == NB - 1),
                    )
                y_t = ypool.tile([P, ck], io_dt, tag="y")
                nc.vector.tensor_copy(y_t[:], py[:])
                nc.sync.dma_start(y_r[:, dn, off:off + ck], y_t[:])

    nc.compile()
    return nc, CT


def _get_nc(counts):
    key = tuple(counts)
    if key not in _cache:
        _cache[key] = _build_bass(counts)
    return _cache[key]


def _route(x, W_router):
    """Host-side router: top-2 selection + renormalized weights (fp64).

    Matches jax.lax.top_k on softmax(logits): softmax is monotone so
    top-2 of logits is identical, with ties broken toward lower index
    (argsort stable on -logits).
    """
    lg = x.astype(np.float64) @ W_router.T.astype(np.float64)
    top2 = np.argsort(-lg, axis=1, kind="stable")[:, :TOP_K]
    l1 = np.take_along_axis(lg, top2[:, 0:1], 1)
    l2 = np.take_along_axis(lg, top2[:, 1:2], 1)
    e2 = np.exp(l2 - l1)
    w1 = (1.0 / (1.0 + e2)).astype(np.float32)
    w2 = (e2 / (1.0 + e2)).astype(np.float32)
    return top2, np.concatenate([w1, w2], axis=1)


def kernel(residual, W_router, W_in, b_in, W_out, b_out):
    global LAST_RESULT

    x = np.ascontiguousarray(np.asarray(residual, dtype=np.float32).reshape(T, D))
    W_in = np.asarray(W_in, dtype=np.float32)
    W_out = np.asarray(W_out, dtype=np.float32)
    b_in = np.asarray(b_in, dtype=np.float32)
    b_out = np.asarray(b_out, dtype=np.float32)

    top2, wts = _route(x, np.asarray(W_router, dtype=np.float32))

    idxs, ws = [], []
    for e in range(E):
        sel0 = top2[:, 0] == e
        sel1 = top2[:, 1] == e
        idx = np.concatenate([np.where(sel0)[0], np.where(sel1)[0]])
        w = np.concatenate([wts[sel0, 0], wts[sel1, 1]])
        idxs.append(idx)
        ws.append(w)
    counts = [len(i) for i in idxs]

    nc, CT = _get_nc(counts)

    # One shared token stream: all experts' gathered tokens, transposed
    # to [D, CT] fp16 (the per-expert order matches _chunk_plan's).
    order = np.concatenate(idxs)
    x8 = np.ascontiguousarray(x[order].T.astype(np.float16))
    assert x8.shape[1] == CT

    in_maps = []
    for c in range(NCORES):
        sl = slice(c * FS, (c + 1) * FS)
        in_maps.append({
            "x8": x8,
            "win8": np.ascontiguousarray(
                W_in[:, :, sl].transpose(1, 0, 2).astype(np.float16)),
            "wout8": np.ascontiguousarray(
                W_out[:, sl, :].transpose(1, 0, 2).astype(np.float16)),
            "bin8": np.ascontiguousarray(b_in[:, sl]),
        })

    if os.environ.get("BASS_TRACE"):
        _install_ntff_hook()
    LAST_RESULT = run_bass_kernel_spmd(nc, in_maps, list(range(NCORES)))

    # Host reduction: sum the 8 F-slice partials, add b_out, apply the
    # renormalized top-2 combine weights, scatter-add into [T, D].
    Y = np.zeros((D, CT), dtype=np.float32)
    for c in range(NCORES):
        Y += LAST_RESULT.results[c]["y8"].astype(np.float32)

    y = np.zeros((T, D), dtype=np.float32)
    off = 0
    for e in range(E):
        cnt = counts[e]
        cols = Y[:, off:off + cnt] + b_out[e][:, None]
        y[idxs[e]] += (cols * ws[e][None, :]).T
        off += cnt
    return y.reshape(B, S, D)
